# revision 1
# baseline (speedup 1.0000x reference)
"""GAT v4: snake-packed CSR + replicated-x local tables + batched builds.

Learned from HW microbenchmarks:
- dma_gather cost is DESCRIPTOR-bound (~9ns/slot across 4 SWDGE queues),
  not byte-bound -> keep fat 512B rows [feat|el|er] (el rides for free)
  and attack slot COUNT instead: the membership-preserving snake re-sort
  packs per-tile lo/hi maxima to ~20% padding (120k slots vs 174k).
- per-tile table-build DMAs were SEQ/HWDGE-bound in the timeline sim ->
  build 7 tiles per iteration: one 224KB input DMA, 7 matmuls into 3
  PSUM groups, 3 ACT copies, one strided 7-tile output DMA.
- x is replicated: each core builds the full feat table locally; the only
  collective is a 12.8MB AllGather of transposed bf16 h1.
"""

import sys

sys.path.insert(0, "/opt/trn_rl_repo")

import numpy as np
import ml_dtypes

import concourse.bass as bass
import concourse.bacc as bacc
import concourse.mybir as mybir
from concourse import tile as tile_mod
from concourse import library_config

N = 50000
E = 800000
H = 4
D = 32
HD = H * D
IN_F = 128
NEG_SLOPE = 0.2

NCORES = 8
NT = 49
NPC = NT * 128
NPRIME = NCORES * NPC
NTF = NCORES * NT
LOSPLIT = 32768
HI_OFF = NPRIME - LOSPLIT
DUMLOC = 1407
DUMROW = 5 * NPC + DUMLOC  # 32767
DUMMY_LO = DUMROW
DUMMY_HI = DUMROW - HI_OFF
EL_DUMMY = -150.0

CAP_LO = 40
CAP_HI = 28
TB = 7              # table-build tile batch (divides 49)

F32 = mybir.dt.float32
BF16 = mybir.dt.bfloat16
I16 = mybir.dt.int16
AL = mybir.AluOpType
AF = mybir.ActivationFunctionType


def _plan(src, dst):
    deg = np.bincount(dst, minlength=N)
    order = np.argsort(-deg, kind="stable")
    rank = np.arange(N)
    seq = rank // NCORES
    local = seq + (seq >= DUMLOC)
    rows_by_rank = (rank % NCORES) * NPC + local
    newid0 = np.empty(N, np.int64)
    newid0[order] = rows_by_rank

    # membership-preserving snake re-sort (lo/hi sets stay fixed)
    s2 = newid0[src]
    islo_e = s2 < LOSPLIT
    locnt = np.bincount(dst[islo_e], minlength=N).astype(np.int64)
    hicnt = np.bincount(dst[~islo_e], minlength=N).astype(np.int64)
    assert hicnt.max() < 64 and locnt.max() < 512
    key = locnt * 64 + np.where(locnt % 2 == 0, hicnt, 63 - hicnt)
    is_lo_node = newid0 < LOSPLIT
    row_islo = rows_by_rank < LOSPLIT
    lo_nodes = np.flatnonzero(is_lo_node)
    hi_nodes = np.flatnonzero(~is_lo_node)
    lo_nodes = lo_nodes[np.argsort(-key[lo_nodes], kind="stable")]
    hi_nodes = hi_nodes[np.argsort(-key[hi_nodes], kind="stable")]
    newid = np.empty(N, np.int64)
    newid[lo_nodes] = rows_by_rank[row_islo][:len(lo_nodes)]
    newid[hi_nodes] = rows_by_rank[~row_islo][:len(hi_nodes)]

    s2 = newid[src]
    d2 = newid[dst]
    islo = s2 < LOSPLIT
    assert islo.sum() == islo_e.sum()

    cnt_lo = np.bincount(d2[islo], minlength=NPRIME).reshape(NCORES, NT, 128)
    cnt_hi = np.bincount(d2[~islo], minlength=NPRIME).reshape(NCORES, NT, 128)
    K_lo = np.maximum(cnt_lo.max(axis=(0, 2)).astype(np.int64), 1)
    K_hi = np.maximum(cnt_hi.max(axis=(0, 2)).astype(np.int64), 1)

    okey = d2 * 2 + (~islo)
    sorted_e = np.argsort(okey, kind="stable")
    ok_sorted = okey[sorted_e]
    first = np.r_[True, ok_sorted[1:] != ok_sorted[:-1]]
    runid = np.cumsum(first) - 1
    runstart = np.flatnonzero(first)
    k = np.empty(E, np.int64)
    k[sorted_e] = np.arange(E) - runstart[runid]

    ecore = d2 // NPC
    locd = d2 % NPC
    tl = locd // 128
    pt = locd % 128

    off_lo = np.concatenate([[0], np.cumsum(K_lo)])
    off_hi = np.concatenate([[0], np.cumsum(K_hi)])
    SLO = int(off_lo[-1])
    SHI = int(off_hi[-1])

    idx_lo = np.full((NCORES, SLO * 128), DUMMY_LO, np.int32)
    idx_hi = np.full((NCORES, SHI * 128), DUMMY_HI, np.int32)
    sel = islo
    slot = (off_lo[tl[sel]] + k[sel]) * 128 + pt[sel]
    idx_lo[ecore[sel], slot] = s2[sel]
    sel = ~islo
    slot = (off_hi[tl[sel]] + k[sel]) * 128 + pt[sel]
    idx_hi[ecore[sel], slot] = s2[sel] - HI_OFF

    assert idx_lo.max() < LOSPLIT and idx_hi.max() < LOSPLIT

    def pack(a):
        return np.ascontiguousarray(
            np.tile(a.reshape(-1, 16).T, (8, 1)).astype(np.int16))

    idx_lo_pk = np.stack([pack(idx_lo[c]) for c in range(NCORES)])
    idx_hi_pk = np.stack([pack(idx_hi[c]) for c in range(NCORES)])

    chunks = []
    t0, LO0, HI0, clo, chi = 0, 0, 0, 0, 0
    for t in range(NT):
        if t > t0 and (clo + K_lo[t] > CAP_LO or chi + K_hi[t] > CAP_HI):
            chunks.append((t0, t, LO0, HI0, clo, chi))
            LO0 += clo
            HI0 += chi
            t0, clo, chi = t, 0, 0
        clo += int(K_lo[t])
        chi += int(K_hi[t])
    chunks.append((t0, NT, LO0, HI0, clo, chi))

    return dict(newid=newid, K_lo=K_lo, K_hi=K_hi, chunks=chunks,
                SLO=SLO, SHI=SHI, idx_lo=idx_lo_pk, idx_hi=idx_hi_pk)


def _build_program(plan, nreps=1, p2=True, only="all"):
    K_lo, K_hi, chunks = plan["K_lo"], plan["K_hi"], plan["chunks"]
    SLO, SHI = plan["SLO"], plan["SHI"]
    maxCLO = max(c[4] for c in chunks)
    maxCHI = max(c[5] for c in chunks)

    nc = bacc.Bacc(None, target_bir_lowering=False, debug=False,
                   num_swdge_queues=4)

    xT_full = nc.declare_dram_parameter("xTf", [128, NPRIME], BF16, isOutput=False)
    xT_own = nc.declare_dram_parameter("xT_own", [128, NPC], BF16, isOutput=False)
    W1_in = nc.declare_dram_parameter("W1b", [128, 128], BF16, isOutput=False)
    W2_in = nc.declare_dram_parameter("W2b", [128, 128], BF16, isOutput=False)
    W1T_in = nc.declare_dram_parameter("W1Tb", [128, 128], BF16, isOutput=False)
    W2T_in = nc.declare_dram_parameter("W2Tb", [128, 128], BF16, isOutput=False)
    albd1_in = nc.declare_dram_parameter("albd1", [128, 8], BF16, isOutput=False)
    albd2_in = nc.declare_dram_parameter("albd2", [128, 8], BF16, isOutput=False)
    b1b_in = nc.declare_dram_parameter("b1b", [128, 128], F32, isOutput=False)
    b2b_in = nc.declare_dram_parameter("b2b", [128, 128], F32, isOutput=False)
    dum_in = nc.declare_dram_parameter("dumrow", [1, 136], BF16, isOutput=False)
    ident_in = nc.declare_dram_parameter("ident", [128, 128], BF16, isOutput=False)
    idxlo_in = nc.declare_dram_parameter("idxlo", [128, SLO * 8], I16, isOutput=False)
    idxhi_in = nc.declare_dram_parameter("idxhi", [128, SHI * 8], I16, isOutput=False)
    y_out = nc.declare_dram_parameter("y", [NPC, 32], F32, isOutput=True)

    table1 = nc.dram_tensor("table1", [NPRIME, 256], BF16)
    table2 = nc.dram_tensor("table2", [NPRIME, 256], BF16)
    h1Tshard = nc.dram_tensor("h1Tshard", [128, NPC], BF16)
    h1Tfull = nc.dram_tensor("h1Tfull", [NCORES, 128, NPC], BF16,
                             addr_space="Shared")

    with tile_mod.TileContext(nc) as tc:
        nc.gpsimd.load_library(library_config.mlp)
        with (
            tc.tile_pool(name="const", bufs=1) as cp,
            tc.tile_pool(name="mmin", bufs=3) as mp,
            tc.tile_pool(name="rows", bufs=3) as rp,
            tc.tile_pool(name="psum", bufs=2, space="PSUM") as pp,
            tc.tile_pool(name="psum1", bufs=1, space="PSUM") as pp1,
            tc.tile_pool(name="glo", bufs=2) as glop,
            tc.tile_pool(name="ghi", bufs=2) as ghip,
            tc.tile_pool(name="wk", bufs=2) as wk,
            tc.tile_pool(name="sm", bufs=4) as sm,
        ):
            w1ext = cp.tile([128, 136], BF16)
            w2ext = cp.tile([128, 136], BF16)
            w1t = cp.tile([128, 128], BF16)
            w2t = cp.tile([128, 128], BF16)
            albd1 = cp.tile([128, 8], BF16)
            albd2 = cp.tile([128, 8], BF16)
            b1b = cp.tile([128, 128], F32)
            b2b = cp.tile([128, 128], F32)
            ident = cp.tile([128, 128], BF16)
            idxlo = cp.tile([128, SLO * 8], I16)
            idxhi = cp.tile([128, SHI * 8], I16)
            er1 = cp.tile([128, NT * 4], F32)
            er2 = cp.tile([128, NT * 4], F32)
            war2 = cp.tile([128, 4], BF16)
            h_buf = cp.tile([128, NT * 128], BF16)

            nc.sync.dma_start(out=w1ext[:, 0:128], in_=W1_in[:])
            nc.sync.dma_start(out=w2ext[:, 0:128], in_=W2_in[:])
            nc.sync.dma_start(out=w1t[:], in_=W1T_in[:])
            nc.sync.dma_start(out=w2t[:], in_=W2T_in[:])
            nc.sync.dma_start(out=albd1[:], in_=albd1_in[:])
            nc.sync.dma_start(out=albd2[:], in_=albd2_in[:])
            nc.sync.dma_start(out=b1b[:], in_=b1b_in[:])
            nc.sync.dma_start(out=b2b[:], in_=b2b_in[:])
            nc.sync.dma_start(out=ident[:], in_=ident_in[:])
            nc.sync.dma_start(out=idxlo[:], in_=idxlo_in[:])
            nc.sync.dma_start(out=idxhi[:], in_=idxhi_in[:])

            for wext, wt, albd in ((w1ext, w1t, albd1), (w2ext, w2t, albd2)):
                ps8 = pp1.tile([128, 8], F32, tag="ps8")
                nc.tensor.matmul(ps8[:], lhsT=wt[:], rhs=albd[:],
                                 start=True, stop=True)
                nc.vector.tensor_copy(wext[:, 128:136], ps8[:])
                if wext is w2ext:
                    nc.vector.tensor_copy(war2[:], ps8[:, 4:8])

            def build_full_table(wext, table, src_get7, dum_in):
                """7 tiles per iteration: 1 in-DMA, 7 matmuls, 3 ACT
                copies, 1 strided out-DMA covering 7*128 rows."""
                for t0 in range(0, NTF, TB):
                    lh7 = src_get7(t0)
                    row7 = rp.tile([128, TB, 256], BF16, tag="row")
                    for (ga, gb) in ((0, 3), (3, 6), (6, 7)):
                        ps = pp.tile([128, 408], F32, tag="psg")
                        for i in range(ga, gb):
                            nc.tensor.matmul(
                                ps[:, (i - ga) * 136:(i - ga + 1) * 136],
                                lhsT=lh7[:, i * 128:(i + 1) * 128],
                                rhs=wext[:, 0:136], start=True, stop=True)
                        for i in range(ga, gb):
                            nc.scalar.activation(
                                row7[:, i, 0:136],
                                ps[:, (i - ga) * 136:(i - ga + 1) * 136],
                                AF.Copy)
                    # full 512B-row contiguous writes (cols 136:256 are
                    # garbage the gather never reads); one DMA per tile so
                    # each transfer is a dense 64KB block
                    for i in range(TB):
                        eng = (nc.sync, nc.scalar)[i % 2]
                        eng.dma_start(
                            out=table[(t0 + i) * 128:(t0 + i + 1) * 128, :],
                            in_=row7[:, i, :])
                drow = rp.tile([1, 136], BF16, tag="drow")
                nc.sync.dma_start(out=drow[:], in_=dum_in[:])
                nc.sync.dma_start(out=table[DUMROW:DUMROW + 1, 0:136],
                                  in_=drow[:])

            def src_x7(t0):
                lh7 = mp.tile([128, TB * 128], BF16, tag="lh")
                nc.sync.dma_start(
                    out=lh7[:], in_=xT_full[:, t0 * 128:(t0 + TB) * 128])
                return lh7

            def src_h7(t0):
                c, lt = t0 // NT, t0 % NT
                lh7 = mp.tile([128, TB * 128], BF16, tag="lh")
                nc.sync.dma_start(
                    out=lh7[:],
                    in_=h1Tfull[c, :, lt * 128:(lt + TB) * 128])
                return lh7

            def edge_phase(table, er_t, layer):
                qn = [0]

                def next_q():
                    qn[0] = (qn[0] + 1) % 4
                    return qn[0]

                for (t0, t1, LO0, HI0, clo, chi) in chunks:
                    glo = glop.tile([128, maxCLO, 256], BF16, tag="glo")
                    ghi = ghip.tile([128, maxCHI, 256], BF16, tag="ghi")
                    nc.gpsimd.dma_gather(
                        out_ap=glo[:, 0:clo, :],
                        in_ap=table[0:LOSPLIT, :],
                        idxs_ap=idxlo[:, LO0 * 8:(LO0 + clo) * 8],
                        num_idxs=clo * 128, num_idxs_reg=clo * 128,
                        elem_size=256, single_packet=False, queue_num=next_q(),
                    )
                    nc.gpsimd.dma_gather(
                        out_ap=ghi[:, 0:chi, :],
                        in_ap=table[HI_OFF:NPRIME, :],
                        idxs_ap=idxhi[:, HI0 * 8:(HI0 + chi) * 8],
                        num_idxs=chi * 128, num_idxs_reg=chi * 128,
                        elem_size=256, single_packet=False, queue_num=next_q(),
                    )
                    a = b = 0
                    for t in range(t0, t1):
                        KL, KH = int(K_lo[t]), int(K_hi[t])
                        KT = KL + KH
                        er_ap = er_t[:, t * 4:(t + 1) * 4].unsqueeze(1)
                        q_t = wk.tile([128, KT, 4], F32, tag="q")
                        nc.vector.tensor_tensor(
                            out=q_t[:, 0:KL, :],
                            in0=glo[:, a:a + KL, 128:132],
                            in1=er_ap.broadcast_to([128, KL, 4]), op=AL.add)
                        nc.vector.tensor_tensor(
                            out=q_t[:, KL:KT, :],
                            in0=ghi[:, b:b + KH, 128:132],
                            in1=er_ap.broadcast_to([128, KH, 4]), op=AL.add)
                        e_t = wk.tile([128, KT, 4], F32, tag="e")
                        nc.vector.scalar_tensor_tensor(
                            out=e_t[:], in0=q_t[:], scalar=NEG_SLOPE,
                            in1=q_t[:], op0=AL.mult, op1=AL.max)
                        if p2:
                            pe_t = wk.tile([128, KT, 4], F32, tag="p2")
                            nc.scalar.activation(pe_t[:], e_t[:], AF.Exp)
                            dn = sm.tile([128, 4], F32, tag="dn")
                            nc.vector.reduce_sum(
                                out=dn[:],
                                in_=pe_t.rearrange("p c h -> p h c"),
                                axis=mybir.AxisListType.X)
                            in1lo = pe_t[:, 0:KL, :].unsqueeze(3)\
                                .broadcast_to([128, KL, 4, 32])
                            in1hi = pe_t[:, KL:KT, :].unsqueeze(3)\
                                .broadcast_to([128, KH, 4, 32])
                        else:
                            pbig = wk.tile([128, KT, 128], BF16, tag="pbig")
                            nc.scalar.activation(
                                pbig.rearrange("p c (h d) -> p c h d", h=4),
                                e_t[:].unsqueeze(3).broadcast_to(
                                    [128, KT, 4, 32]),
                                AF.Exp)
                            dn = sm.tile([128, 4], F32, tag="dn")
                            nc.vector.reduce_sum(
                                out=dn[:],
                                in_=pbig.rearrange(
                                    "p c (h d) -> p h d c", h=4)[:, :, 0, :],
                                axis=mybir.AxisListType.X)
                            in1lo = pbig[:, 0:KL, :]\
                                .rearrange("p c (h d) -> p c h d", h=4)
                            in1hi = pbig[:, KL:KT, :]\
                                .rearrange("p c (h d) -> p c h d", h=4)
                        rc = sm.tile([128, 4], F32, tag="rc")
                        nc.vector.reciprocal(rc[:], dn[:])

                        rst = sm.tile([128, 4, 32], F32, tag="rst")
                        w_t = wk.tile([128, KT, 128], BF16, tag="w")
                        w4 = w_t.rearrange("p c (h d) -> p c h d", h=4)
                        nc.vector.tensor_tensor(
                            out=w4[:, 0:KL, :, :],
                            in0=glo[:, a:a + KL, 0:128].rearrange(
                                "p c (h d) -> p c h d", h=4),
                            in1=in1lo, op=AL.mult)
                        nc.vector.tensor_tensor(
                            out=w4[:, KL:KT, :, :],
                            in0=ghi[:, b:b + KH, 0:128].rearrange(
                                "p c (h d) -> p c h d", h=4),
                            in1=in1hi, op=AL.mult)
                        nc.vector.reduce_sum(
                            out=rst[:],
                            in_=w_t.rearrange("p c (h d) -> p h d c", h=4),
                            axis=mybir.AxisListType.X)
                        o_t = sm.tile([128, 4, 32], F32, tag="o")
                        nc.vector.tensor_tensor(
                            out=o_t[:], in0=rst[:],
                            in1=rc[:].unsqueeze(2).broadcast_to([128, 4, 32]),
                            op=AL.mult)
                        flat_o = o_t.rearrange("p h d -> p (h d)")
                        if layer == 1:
                            nc.vector.tensor_tensor(
                                out=flat_o, in0=flat_o, in1=b1b[:], op=AL.add)
                            nc.scalar.activation(
                                h_buf[:, t * 128:(t + 1) * 128], flat_o,
                                AF.Relu)
                            psT = pp.tile([128, 128], BF16, tag="psT")
                            nc.tensor.transpose(
                                psT[:], h_buf[:, t * 128:(t + 1) * 128],
                                ident[:])
                            h1t = mp.tile([128, 128], BF16, tag="h1t")
                            nc.vector.tensor_copy(h1t[:], psT[:])
                            nc.sync.dma_start(
                                out=h1Tshard[:, t * 128:(t + 1) * 128],
                                in_=h1t[:])
                            ps4 = pp.tile([128, 4], F32, tag="ps4")
                            nc.tensor.matmul(ps4[:], lhsT=h1t[:], rhs=war2[:],
                                             start=True, stop=True)
                            nc.vector.tensor_copy(
                                er2[:, t * 4:(t + 1) * 4], ps4[:])
                        else:
                            nc.vector.tensor_tensor(
                                out=flat_o, in0=flat_o, in1=b2b[:], op=AL.add)
                            hf = sm.tile([128, 128], F32, tag="hf")
                            nc.vector.tensor_copy(
                                hf[:], h_buf[:, t * 128:(t + 1) * 128])
                            nc.vector.tensor_tensor(
                                out=flat_o, in0=flat_o, in1=hf[:], op=AL.add)
                            yt = sm.tile([128, 32], F32, tag="yt")
                            nc.vector.reduce_sum(
                                out=yt[:],
                                in_=o_t.rearrange("p h d -> p d h"),
                                axis=mybir.AxisListType.X)
                            nc.scalar.mul(yt[:], yt[:], 0.25)
                            nc.sync.dma_start(
                                out=y_out[t * 128:(t + 1) * 128, :], in_=yt[:])
                        a += KL
                        b += KH

            # er1 via small batched matmuls from xT_own against W1@ar1bd
            war1 = cp.tile([128, 4], BF16)
            ps8 = pp1.tile([128, 8], F32, tag="ps8")
            nc.tensor.matmul(ps8[:], lhsT=w1t[:], rhs=albd1[:],
                             start=True, stop=True)
            nc.vector.tensor_copy(war1[:], ps8[:, 4:8])

            def make_er1():
                for t0 in range(0, NT, TB):
                    lh7 = mp.tile([128, TB * 128], BF16, tag="lho")
                    nc.sync.dma_start(
                        out=lh7[:], in_=xT_own[:, t0 * 128:(t0 + TB) * 128])
                    ps = pp1.tile([128, TB * 4], F32, tag="pse")
                    for i in range(TB):
                        nc.tensor.matmul(
                            ps[:, i * 4:(i + 1) * 4],
                            lhsT=lh7[:, i * 128:(i + 1) * 128],
                            rhs=war1[:], start=True, stop=True)
                    nc.vector.tensor_copy(er1[:, t0 * 4:(t0 + TB) * 4], ps[:])

            def coll():
                nc.gpsimd.collective_compute(
                    "AllGather", AL.bypass,
                    replica_groups=[list(range(NCORES))],
                    ins=[h1Tshard[:, :].opt()],
                    outs=[h1Tfull[:, :, :].opt()],
                )

            if only != "all":
                build_full_table(w1ext, table1, src_x7, dum_in)
                make_er1()
                edge_phase(table1, er1, layer=1)
                for rep in range(nreps):
                    if only == "build":
                        build_full_table(w1ext, table1, src_x7, dum_in)
                    elif only == "edge":
                        edge_phase(table1, er1, layer=1)
                    elif only == "coll":
                        coll()
                # keep outputs defined
                coll() if only != "coll" else None
                build_full_table(w2ext, table2, src_h7, dum_in)
                edge_phase(table2, er2, layer=2)
            else:
                for rep in range(nreps):
                    build_full_table(w1ext, table1, src_x7, dum_in)
                    make_er1()
                    edge_phase(table1, er1, layer=1)
                    coll()
                    build_full_table(w2ext, table2, src_h7, dum_in)
                    edge_phase(table2, er2, layer=2)

    nc.compile()
    return nc


_CACHE = {}


def _get_program_and_plan(src, dst):
    key = (src.tobytes()[:256], dst.tobytes()[:256], src.shape[0])
    if key not in _CACHE:
        plan = _plan(np.asarray(src), np.asarray(dst))
        prog = _build_program(plan)
        _CACHE[key] = (plan, prog)
    return _CACHE[key]


def _make_in_maps(plan, inputs):
    newid = plan["newid"]
    x = np.asarray(inputs["x"])

    def bd(al, ar):
        m = np.zeros((HD, 8), np.float32)
        for h in range(H):
            m[h * D:(h + 1) * D, h] = al[h]
            m[h * D:(h + 1) * D, 4 + h] = ar[h]
        return m.astype(ml_dtypes.bfloat16)

    xP = np.zeros((NPRIME, IN_F), np.float32)
    xP[newid] = x
    xTf = np.ascontiguousarray(xP.T.astype(ml_dtypes.bfloat16))
    xPc = xP.reshape(NCORES, NPC, IN_F)

    dumrow = np.zeros((1, 136), np.float32)
    dumrow[0, 128:132] = EL_DUMMY

    common = {
        "xTf": xTf,
        "W1b": np.asarray(inputs["W1"]).astype(ml_dtypes.bfloat16),
        "W2b": np.asarray(inputs["W2"]).astype(ml_dtypes.bfloat16),
        "W1Tb": np.asarray(inputs["W1"]).T.astype(ml_dtypes.bfloat16).copy(),
        "W2Tb": np.asarray(inputs["W2"]).T.astype(ml_dtypes.bfloat16).copy(),
        "albd1": bd(np.asarray(inputs["al1"]), np.asarray(inputs["ar1"])),
        "albd2": bd(np.asarray(inputs["al2"]), np.asarray(inputs["ar2"])),
        "b1b": np.tile(np.asarray(inputs["b1"])[None, :], (128, 1)).astype(
            np.float32),
        "b2b": np.tile(np.asarray(inputs["b2"])[None, :], (128, 1)).astype(
            np.float32),
        "dumrow": dumrow.astype(ml_dtypes.bfloat16),
        "ident": np.eye(128, dtype=ml_dtypes.bfloat16),
    }
    in_maps = []
    for c in range(NCORES):
        m = dict(common)
        m["xT_own"] = np.ascontiguousarray(
            xPc[c].T.astype(ml_dtypes.bfloat16))
        m["idxlo"] = plan["idx_lo"][c]
        m["idxhi"] = plan["idx_hi"][c]
        in_maps.append(m)
    return in_maps


def _build_null_program(plan):
    SLO, SHI = plan["SLO"], plan["SHI"]
    nc = bacc.Bacc(None, target_bir_lowering=False, debug=False)
    nc.declare_dram_parameter("xTf", [128, NPRIME], BF16, isOutput=False)
    nc.declare_dram_parameter("xT_own", [128, NPC], BF16, isOutput=False)
    nc.declare_dram_parameter("W1b", [128, 128], BF16, isOutput=False)
    nc.declare_dram_parameter("W2b", [128, 128], BF16, isOutput=False)
    nc.declare_dram_parameter("W1Tb", [128, 128], BF16, isOutput=False)
    nc.declare_dram_parameter("W2Tb", [128, 128], BF16, isOutput=False)
    nc.declare_dram_parameter("albd1", [128, 8], BF16, isOutput=False)
    nc.declare_dram_parameter("albd2", [128, 8], BF16, isOutput=False)
    b1b_in = nc.declare_dram_parameter("b1b", [128, 128], F32, isOutput=False)
    nc.declare_dram_parameter("b2b", [128, 128], F32, isOutput=False)
    nc.declare_dram_parameter("dumrow", [1, 136], BF16, isOutput=False)
    nc.declare_dram_parameter("ident", [128, 128], BF16, isOutput=False)
    nc.declare_dram_parameter("idxlo", [128, SLO * 8], I16, isOutput=False)
    nc.declare_dram_parameter("idxhi", [128, SHI * 8], I16, isOutput=False)
    y_out = nc.declare_dram_parameter("y", [NPC, 32], F32, isOutput=True)
    with tile_mod.TileContext(nc) as tc:
        with tc.tile_pool(name="p", bufs=1) as p:
            t = p.tile([128, 32], F32)
            nc.sync.dma_start(out=t[:], in_=b1b_in[:, 0:32])
            nc.sync.dma_start(out=y_out[0:128, :], in_=t[:])
    nc.compile()
    return nc


def kernel(x, src, dst, W1, al1, ar1, b1, W2, al2, ar2, b2):
    src = np.asarray(src)
    dst = np.asarray(dst)
    plan, nc = _get_program_and_plan(src, dst)
    in_maps = _make_in_maps(plan, dict(
        x=x, W1=W1, al1=al1, ar1=ar1, b1=b1,
        W2=W2, al2=al2, ar2=ar2, b2=b2))

    from concourse.bass_utils import run_bass_kernel_spmd
    res = run_bass_kernel_spmd(nc, in_maps, core_ids=list(range(NCORES)))

    y = np.stack([res.results[c]["y"] for c in range(NCORES)])
    out = y.reshape(NPRIME, 32)[plan["newid"]]
    return out.astype(np.float32)

